# revision 7
# baseline (speedup 1.0000x reference)
"""Trainium2 Bass kernel for per-edge-type linear routing (MoE-style).

Computes out[i] = W[type_i] @ x[i] + b[type_i] for N=131072 edges,
C=D=256, T=8 types, on 8 NeuronCores.

Strategy: expert-grouped data parallelism. On the host we stable-sort the
edges by type and deal them round-robin to the 8 cores, so every core gets
~N/8 edges grouped into 8 contiguous per-type segments (segment sizes are
shared across cores, padded to multiples of 128 -> identical SPMD program,
balanced for any type distribution). Each core runs ONE dense GEMM pass
over its columns: for each 128-edge tile the stationary operand is the
transposed edge-feature tile and the moving operand is the type's
transposed weight matrix; bias is added by the DVE during the PSUM->SBUF
evict. This does 1x the required FLOPs (the reference's masked form does
T=8x) and runs at the fp32 PE roofline (4 cycles/column).

Device layout per core:
  xt   [256, NP]  fp32  edge features, transposed, type-grouped + padded
  wt   [8, 256, 256] fp32  wt[t] = W[t].T (contraction dim first)
  bias [8, 256]   fp32
  y    [NP, 256]  fp32  outputs in the same grouped order

The host then scatters valid rows of y back to the original edge order.
"""

import numpy as np

N_CORES = 8
T = 8
C = 256
D = 256
P = 128
BLK = 512  # columns processed per DMA block (4 n-tiles of 128)

_cache = {}


def _build_program(NP, tile_type):
    """Build + compile the SPMD Bass program for one core.

    NP: padded number of edge columns (multiple of BLK).
    tile_type: tuple of per-128-column-tile type ids, len NP // P.
    """
    import concourse.tile as tile
    from concourse import bacc, mybir

    f32 = mybir.dt.float32
    nc = bacc.Bacc("TRN2", target_bir_lowering=False, debug=False)

    xt = nc.dram_tensor("xt", [C, NP], f32, kind="ExternalInput")
    wt = nc.dram_tensor("wt", [T, C, D], f32, kind="ExternalInput")
    bias = nc.dram_tensor("bias", [T, D], f32, kind="ExternalInput")
    chain = nc.dram_tensor("chain", [1, 4], f32, kind="ExternalInput")
    y = nc.dram_tensor("y", [NP, D], f32, kind="ExternalOutput")
    chain_out = nc.dram_tensor("chain_out", [1, 4], f32, kind="ExternalOutput")

    n_blocks = NP // BLK
    tiles_per_blk = BLK // P

    with tile.TileContext(nc) as tc:
        with (
            tc.tile_pool(name="wpool", bufs=1) as wpool,
            tc.tile_pool(name="xpool", bufs=6) as xpool,
            tc.tile_pool(name="opool", bufs=6) as opool,
            tc.tile_pool(name="pspool", bufs=8, space="PSUM") as pspool,
        ):
            # Tiny passthrough so a timing harness can chain executions.
            cht = wpool.tile([1, 4], f32, name="cht", tag="cht")
            nc.sync.dma_start(out=cht[:], in_=chain[:])
            nc.sync.dma_start(out=chain_out[:], in_=cht[:])

            # Weight / bias tiles are loaded lazily right before first use so
            # they do not delay the first x blocks on the DMA engines.
            wtiles = {}

            def ensure_w(t):
                if t in wtiles:
                    return
                halves = []
                for h in range(2):
                    w_ = wpool.tile([P, D], f32, name=f"w{t}_{h}", tag=f"w{t}_{h}")
                    nc.sync.dma_start(out=w_[:], in_=wt[t, h * P:(h + 1) * P, :])
                    halves.append(w_)
                # bias row broadcast across partitions; added during evict
                bt = wpool.tile([P, D], f32, name=f"b{t}", tag=f"b{t}")
                nc.sync.dma_start(
                    out=bt[:], in_=bias[t:t + 1, :].to_broadcast((P, D))
                )
                wtiles[t] = (halves, bt)

            for blk in range(n_blocks):
                c0 = blk * BLK
                xb0 = xpool.tile([P, BLK], f32, name="xb0", tag="xb0")
                xb1 = xpool.tile([P, BLK], f32, name="xb1", tag="xb1")
                nc.sync.dma_start(out=xb0[:], in_=xt[0:P, c0:c0 + BLK])
                nc.sync.dma_start(out=xb1[:], in_=xt[P:C, c0:c0 + BLK])
                ost = opool.tile([P, tiles_per_blk * D], f32, name="ost", tag="ost")
                for j in range(tiles_per_blk):
                    t = tile_type[blk * tiles_per_blk + j]
                    ensure_w(t)
                    halves, bt = wtiles[t]
                    ps = pspool.tile([P, D], f32, name="ps", tag="ps")
                    nc.tensor.matmul(
                        ps[:], xb0[:, j * P:(j + 1) * P], halves[0][:],
                        start=True, stop=False,
                    )
                    nc.tensor.matmul(
                        ps[:], xb1[:, j * P:(j + 1) * P], halves[1][:],
                        start=False, stop=True,
                    )
                    nc.vector.tensor_tensor(
                        ost[:, j * D:(j + 1) * D], ps[:], bt[:],
                        op=mybir.AluOpType.add,
                    )
                out_ap = y[c0:c0 + BLK, :].rearrange("(j p) e -> p j e", p=P)
                nc.sync.dma_start(out=out_ap, in_=ost[:])

    nc.compile()
    return nc


def _plan(ids):
    """Shared sharding plan: returns (core_idx, offs, G, NP_pad, tile_type)."""
    order = np.argsort(ids, kind="stable")
    core_idx = [order[k::N_CORES] for k in range(N_CORES)]
    cnts = np.stack(
        [np.bincount(ids[ci], minlength=T).astype(np.int64) for ci in core_idx]
    )
    gmax = cnts.max(axis=0)
    G = ((gmax + P - 1) // P) * P
    NP = int(G.sum())
    NP_pad = ((NP + BLK - 1) // BLK) * BLK
    offs = np.concatenate([[0], np.cumsum(G)]).astype(np.int64)
    tile_type = []
    for t in range(T):
        tile_type += [t] * (int(G[t]) // P)
    tile_type += [0] * ((NP_pad - NP) // P)
    return core_idx, offs, tuple(tile_type), NP_pad


def kernel(edge_features, weights, biases, edge_type_ids):
    from concourse.bass_utils import run_bass_kernel_spmd

    x = np.ascontiguousarray(np.asarray(edge_features), dtype=np.float32)
    w = np.ascontiguousarray(np.asarray(weights), dtype=np.float32)
    b = np.ascontiguousarray(np.asarray(biases), dtype=np.float32)
    ids = np.asarray(edge_type_ids)
    n = x.shape[0]

    core_idx, offs, tile_type, NP_pad = _plan(ids)

    key = (NP_pad, tile_type)
    if key not in _cache:
        _cache[key] = _build_program(NP_pad, tile_type)
    nc = _cache[key]

    wt_full = np.ascontiguousarray(w.transpose(0, 2, 1))  # [T, C, D]
    chain0 = np.zeros((1, 4), dtype=np.float32)
    in_maps = []
    seg_rows = []
    for k in range(N_CORES):
        ci = core_idx[k]
        ids_k = ids[ci]
        xr = np.zeros((NP_pad, C), dtype=np.float32)
        segs = []
        for t in range(T):
            idx_t = ci[ids_k == t]
            cnt = idx_t.shape[0]
            if cnt:
                xr[offs[t]:offs[t] + cnt] = x[idx_t]
            segs.append((int(offs[t]), cnt, idx_t))
        seg_rows.append(segs)
        in_maps.append({
            "xt": np.ascontiguousarray(xr.T),
            "wt": wt_full,
            "bias": b,
            "chain": chain0,
        })

    res = run_bass_kernel_spmd(nc, in_maps, list(range(N_CORES)))

    out = np.empty((n, D), dtype=np.float32)
    for k in range(N_CORES):
        yk = res.results[k]["y"]
        for off, cnt, idx_t in seg_rows[k]:
            if cnt:
                out[idx_t] = yk[off:off + cnt]
    return out


# revision 9
# speedup vs baseline: 1550.3991x; 1550.3991x over previous
"""Trainium2 Bass kernel for per-edge-type linear routing (MoE-style).

Computes out[i] = W[type_i] @ x[i] + b[type_i] for N=131072 edges,
C=D=256, T=8 types, on 8 NeuronCores.

Strategy: expert-grouped data parallelism. On the host we stable-sort the
edges by type and deal them round-robin to the 8 cores, so every core gets
~N/8 edges grouped into 8 contiguous per-type segments (segment sizes are
shared across cores, padded to multiples of 128 -> identical SPMD program,
balanced for any type distribution). Each core runs ONE dense GEMM pass
over its columns: for each 128-edge tile the stationary operand is the
transposed edge-feature tile and the moving operand is the type's
transposed weight matrix; bias is added by the DVE during the PSUM->SBUF
evict. This does 1x the required FLOPs (the reference's masked form does
T=8x) and runs at the fp32 PE roofline (4 cycles/column).

Device layout per core:
  xt   [256, NP]  fp32  edge features, transposed, type-grouped + padded
  wt   [8, 256, 256] fp32  wt[t] = W[t].T (contraction dim first)
  bias [8, 256]   fp32
  y    [NP, 256]  fp32  outputs in the same grouped order

The host then scatters valid rows of y back to the original edge order.
"""

import numpy as np

N_CORES = 8
T = 8
C = 256
D = 256
P = 128
BLK = 512  # columns processed per DMA block (4 n-tiles of 128)

_cache = {}


def _build_program(NP, tile_type):
    """Build + compile the SPMD Bass program for one core.

    NP: padded number of edge columns (multiple of BLK).
    tile_type: tuple of per-128-column-tile type ids, len NP // P.
    """
    import concourse.tile as tile
    from concourse import bacc, mybir

    f32 = mybir.dt.float32
    nc = bacc.Bacc("TRN2", target_bir_lowering=False, debug=False)

    xt = nc.dram_tensor("xt", [C, NP], f32, kind="ExternalInput")
    wt = nc.dram_tensor("wt", [T, C, D], f32, kind="ExternalInput")
    bias = nc.dram_tensor("bias", [T, D], f32, kind="ExternalInput")
    chain = nc.dram_tensor("chain", [1, 4], f32, kind="ExternalInput")
    y = nc.dram_tensor("y", [NP, D], f32, kind="ExternalOutput")
    chain_out = nc.dram_tensor("chain_out", [1, 4], f32, kind="ExternalOutput")

    n_blocks = NP // BLK
    tiles_per_blk = BLK // P

    with tile.TileContext(nc) as tc:
        with (
            tc.tile_pool(name="wpool", bufs=1) as wpool,
            tc.tile_pool(name="xpool", bufs=6) as xpool,
            tc.tile_pool(name="opool", bufs=6) as opool,
            tc.tile_pool(name="pspool", bufs=8, space="PSUM") as pspool,
        ):
            # Tiny passthrough so a timing harness can chain executions.
            cht = wpool.tile([1, 4], f32, name="cht", tag="cht")
            nc.sync.dma_start(out=cht[:], in_=chain[:])
            nc.sync.dma_start(out=chain_out[:], in_=cht[:])

            # Weight / bias tiles are loaded lazily right before first use so
            # they do not delay the first x blocks on the DMA engines.
            wtiles = {}

            def ensure_w(t):
                if t in wtiles:
                    return
                halves = []
                for h in range(2):
                    w_ = wpool.tile([P, D], f32, name=f"w{t}_{h}", tag=f"w{t}_{h}")
                    nc.sync.dma_start(out=w_[:], in_=wt[t, h * P:(h + 1) * P, :])
                    halves.append(w_)
                # bias row broadcast across partitions; added during evict
                bt = wpool.tile([P, D], f32, name=f"b{t}", tag=f"b{t}")
                nc.sync.dma_start(
                    out=bt[:], in_=bias[t:t + 1, :].to_broadcast((P, D))
                )
                wtiles[t] = (halves, bt)

            for blk in range(n_blocks):
                c0 = blk * BLK
                xb0 = xpool.tile([P, BLK], f32, name="xb0", tag="xb0")
                xb1 = xpool.tile([P, BLK], f32, name="xb1", tag="xb1")
                nc.sync.dma_start(out=xb0[:], in_=xt[0:P, c0:c0 + BLK])
                nc.sync.dma_start(out=xb1[:], in_=xt[P:C, c0:c0 + BLK])
                ost = opool.tile([P, tiles_per_blk * D], f32, name="ost", tag="ost")
                for j in range(tiles_per_blk):
                    t = tile_type[blk * tiles_per_blk + j]
                    ensure_w(t)
                    halves, bt = wtiles[t]
                    ps = pspool.tile([P, D], f32, name="ps", tag="ps")
                    nc.tensor.matmul(
                        ps[:], xb0[:, j * P:(j + 1) * P], halves[0][:],
                        start=True, stop=False,
                    )
                    nc.tensor.matmul(
                        ps[:], xb1[:, j * P:(j + 1) * P], halves[1][:],
                        start=False, stop=True,
                    )
                    nc.vector.tensor_tensor(
                        ost[:, j * D:(j + 1) * D], ps[:], bt[:],
                        op=mybir.AluOpType.add,
                    )
                out_ap = y[c0:c0 + BLK, :].rearrange("(j p) e -> p j e", p=P)
                nc.sync.dma_start(out=out_ap, in_=ost[:])

    nc.compile()
    return nc


def _plan(ids):
    """Shared sharding plan: returns (core_idx, offs, G, NP_pad, tile_type)."""
    order = np.argsort(ids, kind="stable")
    core_idx = [order[k::N_CORES] for k in range(N_CORES)]
    cnts = np.stack(
        [np.bincount(ids[ci], minlength=T)[:T].astype(np.int64)
         for ci in core_idx]
    )
    gmax = cnts.max(axis=0)
    G = ((gmax + P - 1) // P) * P
    NP = int(G.sum())
    NP_pad = ((NP + BLK - 1) // BLK) * BLK
    offs = np.concatenate([[0], np.cumsum(G)]).astype(np.int64)
    tile_type = []
    for t in range(T):
        tile_type += [t] * (int(G[t]) // P)
    tile_type += [0] * ((NP_pad - NP) // P)
    return core_idx, offs, tuple(tile_type), NP_pad


def kernel(edge_features, weights, biases, edge_type_ids):
    from concourse.bass_utils import run_bass_kernel_spmd

    x = np.ascontiguousarray(np.asarray(edge_features), dtype=np.float32)
    w = np.ascontiguousarray(np.asarray(weights), dtype=np.float32)
    b = np.ascontiguousarray(np.asarray(biases), dtype=np.float32)
    ids = np.asarray(edge_type_ids)
    n = x.shape[0]

    core_idx, offs, tile_type, NP_pad = _plan(ids)

    key = (NP_pad, tile_type)
    if key not in _cache:
        _cache[key] = _build_program(NP_pad, tile_type)
    nc = _cache[key]

    wt_full = np.ascontiguousarray(w.transpose(0, 2, 1))  # [T, C, D]
    chain0 = np.zeros((1, 4), dtype=np.float32)
    in_maps = []
    seg_rows = []
    for k in range(N_CORES):
        ci = core_idx[k]
        ids_k = ids[ci]
        xr = np.zeros((NP_pad, C), dtype=np.float32)
        segs = []
        for t in range(T):
            idx_t = ci[ids_k == t]
            cnt = idx_t.shape[0]
            if cnt:
                xr[offs[t]:offs[t] + cnt] = x[idx_t]
            segs.append((int(offs[t]), cnt, idx_t))
        seg_rows.append(segs)
        in_maps.append({
            "xt": np.ascontiguousarray(xr.T),
            "wt": wt_full,
            "bias": b,
            "chain": chain0,
        })

    res = run_bass_kernel_spmd(nc, in_maps, list(range(N_CORES)))

    # zeros, not empty: rows whose type id falls outside [0, T) are never
    # written by any segment, and the reference leaves them at zero too
    out = np.zeros((n, D), dtype=np.float32)
    for k in range(N_CORES):
        yk = res.results[k]["y"]
        for off, cnt, idx_t in seg_rows[k]:
            if cnt:
                out[idx_t] = yk[off:off + cnt]
    return out
